# revision 1
# baseline (speedup 1.0000x reference)
"""MHA on 8 NeuronCores, v3: query-token-sharded attention, ACT-bound pipeline.

Core c owns token block c = (batch c//2, seq half c%2), 1024 tokens.
The attention phase is ACT(exp)-bound (~262k exp-rows/core is the hard
floor), so everything is scheduled around keeping ACT 100% busy:
  - Phase order: warmup spin (HAM un-throttle) -> proj_v t0-3 -> proj_k ->
    proj_v t4-7 -> proj_q ch0 -> attention with proj_q ch1-7 interleaved as
    PE filler -> wo.
  - Dummy collective at t~0 absorbs the one-time CC rendezvous barrier.
    CC order (deadline-sorted): V-half0, K-halfA, V-half1, K-halfB.
  - Attention step = (head, kc): one scores matmul (n=1024), one exp
    [128,1024], two PV matmuls (n=512) lagged L_PV steps behind scores.
  - PSUM: sg tag 2x[128,1024] (4 banks) + proj tag 1x[128,1024] (2) +
    pv tag 2x[65,512] (2) = 8 banks.
  - wk/wq are consumed one 128-column chunk at a time, so the host passes
    them chunk-major ([8, 128, 1024]) and they stream through small tiles.
  - Softmax denominator from the ones-column of V (65th col); pv psum is
    released via one DVE copy to a_raw; normalization (DVE reciprocal +
    GpSimd partition_broadcast + DVE mul) runs off the critical path.
"""
import numpy as np
import ml_dtypes

import concourse.bass as bass
import concourse.bacc as bacc
import concourse.tile as tile
import concourse.mybir as mybir

N_CORES = 8
P = 128
B, S, D = 4, 2048, 1024
TOK = 1024  # my tokens
CD = D // P  # 8 chunks
NKC = S // P  # 16 key chunks
F32 = mybir.dt.float32
BF16 = mybir.dt.bfloat16
EXP = mybir.ActivationFunctionType.Exp
PAIR_GROUPS = [[2 * i, 2 * i + 1] for i in range(4)]

# processing order of key chunks: V-arrival order (half0 both slots, then half1)
KC_ORDER = [0, 1, 2, 3, 8, 9, 10, 11, 4, 5, 6, 7, 12, 13, 14, 15]
L_PV = 8  # PV lag in steps behind scores/exp

_CACHE = {}


def _n_excess_waits(nc):
    import json

    m = json.loads(nc.to_json_bytes())
    insts = [i for f in m["functions"] for b in f["blocks"] for i in b["instructions"]]
    return sum(
        1
        for i in insts
        if len((i.get("sync_info") or {}).get("on_wait", [])) >= 2
        and i.get("opcode") != "EventSemaphore"
    )


def _finish(nc):
    nc.compile()
    import bass_rust

    for _ in range(6):
        if _n_excess_waits(nc) == 0:
            break
        bass_rust.generate_event_semaphores(nc)
    assert _n_excess_waits(nc) == 0, "excess sync waits remain"
    nc.codegen_inst_isa_subclasses()
    return nc


def build_nc(scopes=False):
    nc = bacc.Bacc("TRN2", target_bir_lowering=False, debug=False, num_devices=N_CORES)

    xqT_d = nc.dram_tensor("xqT", [D, TOK], BF16, kind="ExternalInput").ap()
    xkT_d = nc.dram_tensor("xkT", [D, TOK], BF16, kind="ExternalInput").ap()
    xvT_d = nc.dram_tensor("xvT", [D, TOK], BF16, kind="ExternalInput").ap()
    # wk/wq chunk-major: [out-chunk i, partition p, (j, q)] with
    # wk4[i, p, j*128+q] = wk.T[j*128+p, i*128+q]
    wk4_d = nc.dram_tensor("wk4", [CD, P, D], BF16, kind="ExternalInput").ap()
    wq4_d = nc.dram_tensor("wq4", [CD, P, D], BF16, kind="ExternalInput").ap()
    wvT_d = nc.dram_tensor("wvT", [D, D], BF16, kind="ExternalInput").ap()
    woT_d = nc.dram_tensor("woT", [D, D], BF16, kind="ExternalInput").ap()
    out = nc.dram_tensor("out", [TOK, D], F32, kind="ExternalOutput").ap()

    # exchange buffers
    kag_i = nc.dram_tensor("kag_i", [D, TOK], BF16).ap()
    kag_oA = nc.dram_tensor("kag_oA", [2, D // 2, TOK], BF16).ap()  # d-chunks 0-3
    kag_oB = nc.dram_tensor("kag_oB", [2, D // 2, TOK], BF16).ap()  # d-chunks 4-7
    vag_i = nc.dram_tensor("vag_i", [TOK, D], BF16).ap()
    rec_d = nc.dram_tensor("rec_d", [32, 512], F32).ap()
    # V halves by my-token halves: vag_os[half][slot, tok512, d]
    vag_os = [
        nc.dram_tensor(f"vag_o{h}", [2, TOK // 2, D], BF16).ap() for h in range(2)
    ]

    from contextlib import ExitStack, nullcontext

    def scope(name):
        return nc.named_scope(name) if scopes else nullcontext()

    AG_KW = dict(
        kind="AllGather", op=mybir.AluOpType.bypass, replica_groups=PAIR_GROUPS
    )

    with tile.TileContext(nc) as tc:
        persist = ExitStack()
        qp = persist.enter_context(tc.tile_pool(name="qp", bufs=1))
        ltp = persist.enter_context(tc.tile_pool(name="ltp", bufs=1))
        kst = persist.enter_context(tc.tile_pool(name="kst", bufs=1))
        psp = persist.enter_context(tc.tile_pool(name="psp", bufs=1, space="PSUM"))
        stp = persist.enter_context(tc.tile_pool(name="stp", bufs=1))
        vp = persist.enter_context(tc.tile_pool(name="vp", bufs=1))
        pgp = persist.enter_context(tc.tile_pool(name="pgp", bufs=1))
        smp = persist.enter_context(tc.tile_pool(name="smp", bufs=1))
        arp = persist.enter_context(tc.tile_pool(name="arp", bufs=1))

        # close order is wkx -> wvx -> wqx, so create in reverse (pool stack is LIFO)
        wqx_stack = ExitStack()
        wqx = wqx_stack.enter_context(tc.tile_pool(name="wqx", bufs=1))
        wvx_stack = ExitStack()
        wvx = wvx_stack.enter_context(tc.tile_pool(name="wvx", bufs=1))
        wkx_stack = ExitStack()
        wkx = wkx_stack.enter_context(tc.tile_pool(name="wkx", bufs=1))

        with scope("warm"):
            wtile = smp.tile([128, 512], BF16, name="wtile")
            nc.vector.memset(wtile, 0.001)

        # ---------------- loads (order matters: v, k, q) --------------------
        wk_c = {}  # streamed wk chunk tiles: i -> [128, 8, 128]
        wq_c = {}

        def load_wk_chunk(i):
            t = wkx.tile([P, CD, P], BF16, name=f"wkc_{i}", tag="wkc", bufs=3)
            nc.sync.dma_start(out=t, in_=wk4_d[i].rearrange("p (j q) -> p j q", q=P))
            wk_c[i] = t

        def load_wq_chunk(i):
            t = wqx.tile([P, CD, P], BF16, name=f"wqc_{i}", tag="wqc", bufs=3)
            nc.sync.dma_start(out=t, in_=wq4_d[i].rearrange("p (j q) -> p j q", q=P))
            wq_c[i] = t

        with scope("load"):
            xvT, wv_t, xkT, xqT = [], [], [], []
            for j in range(CD):
                t = wvx.tile([P, TOK], BF16, name=f"xvT_{j}")
                nc.sync.dma_start(out=t, in_=xvT_d[j * P : (j + 1) * P, :])
                xvT.append(t)
                w = wvx.tile([P, D], BF16, name=f"wv_{j}")
                nc.sync.dma_start(out=w, in_=wvT_d[j * P : (j + 1) * P, :])
                wv_t.append(w)
            for j in range(CD):
                t = wkx.tile([P, TOK], BF16, name=f"xkT_{j}")
                nc.sync.dma_start(out=t, in_=xkT_d[j * P : (j + 1) * P, :])
                xkT.append(t)
            load_wk_chunk(0)
            load_wk_chunk(1)
            for j in range(CD):
                t = wqx.tile([P, TOK], BF16, name=f"xqT_{j}")
                nc.sync.dma_start(out=t, in_=xqT_d[j * P : (j + 1) * P, :])
                xqT.append(t)
            load_wq_chunk(0)
            load_wq_chunk(1)

        def mm2(ps, lhsT, rhs, start, stop):
            # ISA caps matmul output at 512 fp32 elements (one PSUM bank):
            # emit two 512-wide matmuls covering a [*, 1024] psum tile
            for hh in range(2):
                nc.tensor.matmul(
                    ps[:, hh * 512 : (hh + 1) * 512],
                    lhsT,
                    rhs[:, hh * 512 : (hh + 1) * 512],
                    start=start,
                    stop=stop,
                )

        # ---------------- warmup spin: un-throttle PE HAM early -------------
        with scope("spin"):
            for k in range(24):
                ps = psp.tile([P, TOK], F32, name="wm", tag="sg", bufs=2)
                nc.tensor.matmul(
                    ps[:, 0:512], wtile[:, 0:128], wtile, start=True, stop=True
                )

        # ---------------- phase 1a: proj_v t0-3 -----------------------------
        def proj_v_chunk(t_i, copy_eng):
            ps = psp.tile([P, D], F32, name="ps_v", tag="sg", bufs=2)
            for j in range(CD):
                mm2(ps, xvT[j][:, t_i * P : (t_i + 1) * P], wv_t[j],
                    start=(j == 0), stop=(j == CD - 1))
            sb = stp.tile([P, D], BF16, name="sb_v", tag="st", bufs=3)
            copy_eng(sb, ps)
            nc.sync.dma_start(out=vag_i[t_i * P : (t_i + 1) * P, :], in_=sb)

        with scope("proj_v_a"):
            for t_i in range(4):
                proj_v_chunk(
                    t_i, nc.scalar.copy if t_i % 2 == 0 else nc.vector.tensor_copy
                )
        with scope("ag_v0"):
            nc.gpsimd.collective_compute(
                ins=[vag_i[0 : TOK // 2, :]], outs=[vag_os[0][:]], **AG_KW
            )

        # ---------------- phase 1b: proj_k ----------------------------------
        def proj_k_chunk(i, copy_eng):
            if i + 2 < CD:
                load_wk_chunk(i + 2)
            ps = psp.tile([P, TOK], F32, name="ps_k", tag="sg", bufs=2)
            wkc = wk_c.pop(i)
            for j in range(CD):
                mm2(ps, wkc[:, j, :], xkT[j], start=(j == 0), stop=(j == CD - 1))
            sb = stp.tile([P, TOK], BF16, name="sb_k", tag="st", bufs=3)
            copy_eng(sb, ps)
            nc.sync.dma_start(out=kag_i[i * P : (i + 1) * P, :], in_=sb)

        with scope("proj_k"):
            for i in range(CD):
                proj_k_chunk(i, nc.scalar.copy if i % 2 == 0 else nc.vector.tensor_copy)
                if i == 3:
                    with scope("ag_kA"):
                        nc.gpsimd.collective_compute(
                            ins=[kag_i[0 : D // 2, :]], outs=[kag_oA[:]], **AG_KW
                        )
        wkx_stack.close()

        # ---------------- phase 1c: proj_v t4-7, then V1 + KB ags -----------
        with scope("proj_v_b"):
            for t_i in range(4, 8):
                proj_v_chunk(
                    t_i, nc.scalar.copy if t_i % 2 == 0 else nc.vector.tensor_copy
                )
        with scope("ag_v1"):
            nc.gpsimd.collective_compute(
                ins=[vag_i[TOK // 2 : TOK, :]], outs=[vag_os[1][:]], **AG_KW
            )
        with scope("ag_kB"):
            nc.gpsimd.collective_compute(
                ins=[kag_i[D // 2 : D, :]], outs=[kag_oB[:]], **AG_KW
            )
        wvx_stack.close()

        # ---------------- K^T staging into SBUF -----------------------------
        kT_s = [None] * CD

        def kstage(j):
            t = kst.tile([P, S], BF16, name=f"kTs_{j}")
            kg = kag_oA if j < 4 else kag_oB
            jj = j % 4
            nc.sync.dma_start(out=t[:, 0:TOK], in_=kg[0, jj * P : (jj + 1) * P, :])
            nc.sync.dma_start(out=t[:, TOK:S], in_=kg[1, jj * P : (jj + 1) * P, :])
            kT_s[j] = t

        with scope("kstage"):
            for j in range(4):
                kstage(j)

        # ---------------- proj_q ch0 (rest are attn filler) -----------------
        qT_t = [None] * CD

        with scope("proj_q0"):
            ps0 = psp.tile([P, TOK], F32, name="ps_q0", tag="sg", bufs=2)
            wqc0 = wq_c.pop(0)
            for j in range(CD):
                mm2(ps0, wqc0[:, j, :], xqT[j], start=(j == 0), stop=(j == CD - 1))
            qt0 = qp.tile([P, TOK], BF16, name="qT_0")
            nc.scalar.copy(qt0, ps0)
            qT_t[0] = qt0

        with scope("proj_q12"):
            for i in (1, 2):
                load_wq_chunk(i + 1)
                psq = psp.tile([P, TOK], F32, name="ps_q12", tag="sg", bufs=2)
                wqc = wq_c.pop(i)
                for j in range(CD):
                    mm2(psq, wqc[:, j, :], xqT[j], start=(j == 0), stop=(j == CD - 1))
                qt = qp.tile([P, TOK], BF16, name=f"qT_{i}")
                nc.scalar.copy(qt, psq)
                qT_t[i] = qt

        lts = [ltp.tile([P, TOK], BF16, name=f"lt_{i}") for i in range(CD)]

        # ---------------- attention ----------------------------------------
        filler_state = {}
        filler = []
        for i in range(3, CD):
            for j in range(CD):
                def mk(i=i, j=j):
                    def emit():
                        if j == 0:
                            if i + 1 < CD:
                                load_wq_chunk(i + 1)
                            filler_state["ps"] = psp.tile(
                                [P, TOK], F32, name="ps_qf", tag="proj", bufs=1
                            )
                            filler_state["w"] = wq_c.pop(i)
                        mm2(filler_state["ps"], filler_state["w"][:, j, :],
                            xqT[j], start=(j == 0), stop=(j == CD - 1))
                        if j == CD - 1:
                            qt = qp.tile([P, TOK], BF16, name=f"qT_{i}")
                            nc.vector.tensor_copy(qt, filler_state["ps"])
                            qT_t[i] = qt
                    return emit
                filler.append(mk())
        filler.reverse()  # so filler.pop() yields chunk 1 first

        steps = [(h, pos) for h in range(16) for pos in range(NKC)]
        sgs, pgs, pvs, vts = {}, {}, {}, {}

        def load_head(h):
            v_t = vp.tile([P, NKC, 65], BF16, name="v_t", tag="vp", bufs=3)
            for half in range(2):
                for slot in range(2):
                    vsrc = vag_os[half][slot, :, 64 * h : 64 * h + 64]
                    base = 8 * half + 4 * slot
                    nc.sync.dma_start(
                        out=v_t[:, base : base + 4, 0:64],
                        in_=vsrc.rearrange("(kc p) d -> p kc d", p=P),
                    )
            nc.vector.memset(v_t[:, :, 64:65], 1.0)
            vts[h] = v_t
            # vts pos p: half=p//8, slot=(p%8)//4, i=p%4 -> kc = slot*8+half*4+i
            # which equals KC_ORDER[p].

        def emit_scores(s):
            h, pos = steps[s]
            if pos == 0:
                load_head(h)
            kc = KC_ORDER[pos]
            r = slice(64 * (h % 2), 64 * (h % 2) + 64)
            sg = psp.tile([P, TOK], F32, name="sg", tag="sg", bufs=2)
            mm2(sg, kT_s[h // 2][r, kc * P : (kc + 1) * P], qT_t[h // 2][r, :],
                start=True, stop=True)
            sgs[s] = sg

        def emit_exp(s):
            pg = pgp.tile([P, TOK], BF16, name="pg", tag="pg", bufs=L_PV + 2)
            nc.scalar.activation(pg, sgs.pop(s), EXP, scale=0.125)
            pgs[s] = pg

        def emit_pv(s):
            h, pos = steps[s]
            pg = pgs.pop(s)
            if pos == 0:
                pvs[(h, 0)] = psp.tile([65, 512], F32, name="pv0", tag="pv", bufs=2)
                pvs[(h, 1)] = psp.tile([65, 512], F32, name="pv1", tag="pv", bufs=2)
            for qb in range(2):
                nc.tensor.matmul(
                    pvs[(h, qb)],
                    vts[h][:, pos, :],
                    pg[:, qb * 512 : (qb + 1) * 512],
                    start=(pos == 0),
                    stop=(pos == NKC - 1),
                )
            if pos == NKC - 1:
                del vts[h]
                finish_head(h)

        def finish_head(h):
            with scope(f"norm_h{h}"):
                rr = slice(64 * (h % 2), 64 * (h % 2) + 64)
                for qb in range(2):
                    pv = pvs.pop((h, qb))
                    # one DVE copy releases the pv psum bank fast
                    ar = arp.tile([65, 512], F32, name="ar", tag="ar", bufs=2)
                    nc.vector.tensor_copy(ar, pv)
                    rsq = smp.tile([1, 512], F32, name="rsq", tag="rsq", bufs=2)
                    nc.vector.reciprocal(rsq, ar[64:65, :])
                    row = 2 * h + qb
                    nc.sync.dma_start(out=rec_d[row : row + 1, :], in_=rsq)
                    bc = smp.tile([64, 512], F32, name="bc", tag="bc", bufs=2)
                    nc.sync.dma_start(
                        out=bc,
                        in_=bass.AP(
                            tensor=rec_d.tensor,
                            offset=row * 512,
                            ap=[[0, 64], [1, 512]],
                        ),
                    )
                    nc.vector.tensor_mul(
                        lts[h // 2][rr, qb * 512 : (qb + 1) * 512], ar[0:64, :], bc
                    )

        wo_t = []
        with scope("attn"):
            emit_scores(0)
            n = len(steps)
            for s in range(n):
                emit_exp(s)
                if filler:
                    filler.pop()()
                elif s == 40:
                    for j in range(4, CD):
                        kstage(j)
                elif s == 57:
                    # filler drained: free xq/wq space, start wo prefetch there
                    wqx_stack.close()
                    wop = persist.enter_context(tc.tile_pool(name="wop", bufs=1))
                    for j in range(CD):
                        wt3 = wop.tile([P, D], BF16, name=f"wo_{j}")
                        nc.sync.dma_start(out=wt3, in_=woT_d[j * P : (j + 1) * P, :])
                        wo_t.append(wt3)

                if s + 1 < n:
                    emit_scores(s + 1)
                if s >= L_PV:
                    emit_pv(s - L_PV)
            for s in range(n - L_PV, n):
                emit_pv(s)

        # ---------------- phase 3: output projection ------------------------
        with scope("wo"):
            for t_i in range(CD):
                ps3 = psp.tile([P, D], F32, name="ps3", tag="sg", bufs=2)
                for sc in range(CD):
                    mm2(ps3, lts[sc][:, t_i * P : (t_i + 1) * P], wo_t[sc],
                        start=(sc == 0), stop=(sc == CD - 1))
                ob = stp.tile([P, D], F32, name="ob", tag="ob", bufs=2)
                nc.vector.tensor_copy(ob, ps3)
                nc.sync.dma_start(out=out[t_i * P : (t_i + 1) * P, :], in_=ob)

        persist.close()

    return _finish(nc)


def _get_nc(scopes=False):
    key = ("nc", scopes)
    if key not in _CACHE:
        _CACHE[key] = build_nc(scopes)
    return _CACHE[key]


def _chunk_major(wT):
    # wT: [D, D] = w.T ; return [CD, P, D] with out[i, p, j*128+q] = wT[j*128+p, i*128+q]
    return np.ascontiguousarray(
        wT.reshape(CD, P, CD, P).transpose(2, 1, 0, 3).reshape(CD, P, D)
    )


def make_in_maps(query, key, value, wq, wk, wv, wo):
    qf = np.asarray(query, np.float32).reshape(B * S, D)
    kf = np.asarray(key, np.float32).reshape(B * S, D)
    vf = np.asarray(value, np.float32).reshape(B * S, D)
    wk4_h = _chunk_major(np.asarray(wk).T.astype(np.float32)).astype(ml_dtypes.bfloat16)
    wq4_h = _chunk_major(np.asarray(wq).T.astype(np.float32)).astype(ml_dtypes.bfloat16)
    wvT_h = np.ascontiguousarray(np.asarray(wv).T).astype(ml_dtypes.bfloat16)
    woT_h = np.ascontiguousarray(np.asarray(wo).T).astype(ml_dtypes.bfloat16)
    in_maps = []
    for c in range(N_CORES):
        sl = slice(c * TOK, (c + 1) * TOK)
        in_maps.append(
            {
                "xqT": np.ascontiguousarray(qf[sl].T).astype(ml_dtypes.bfloat16),
                "xkT": np.ascontiguousarray(kf[sl].T).astype(ml_dtypes.bfloat16),
                "xvT": np.ascontiguousarray(vf[sl].T).astype(ml_dtypes.bfloat16),
                "wk4": wk4_h,
                "wq4": wq4_h,
                "wvT": wvT_h,
                "woT": woT_h,
            }
        )
    return in_maps


def assemble(results):
    blocks = [results[c]["out"] for c in range(N_CORES)]
    return np.concatenate(blocks, 0).reshape(B, S, D).astype(np.float32)


def kernel(query, key, value, mask, wq, wk, wv, wo):
    # mask is all-False in this problem: softmax without masking.
    nc = _get_nc()
    in_maps = make_in_maps(query, key, value, wq, wk, wv, wo)
    from concourse.bass_utils import run_bass_kernel_spmd

    res = run_bass_kernel_spmd(nc, in_maps, list(range(N_CORES)))
    return assemble(res.results)



# revision 13
# speedup vs baseline: 1.4360x; 1.4360x over previous
"""MHA on 8 NeuronCores, v4: head-pair row-tiled attention, ACT-bound pipeline.

Core c owns token block c = (batch c//2, seq half c%2), 1024 tokens.

Key idea vs v3: the scores matmul has contraction = head_dim = 64, so two
heads (even head at SBUF rows 0-63, odd head at rows 64-127 of kT_s/qT_t)
can stream CONCURRENTLY on the PE array via row tiling (tile_position is
auto-inferred from base partitions; different row_grps overlap on HW).
Attention therefore processes HEAD PAIRS: per pair-step (hp, kc):
  - 4 scores matmuls (A/B alternating row groups, 2 psum tiles
    sg01=[A qb0|B qb0], sg23=[A qb1|B qb1]) — A+B pairs stream together.
  - 2 exps [128,1024] on ACT (the pipeline bottleneck at full clock).
  - 4 PV matmuls (lagged L pair-steps) into 4 pv psum accumulators.
PSUM: sg01 (2 banks) + sg23 (2) + pv 4x[65,512] (4) = 8 banks, so proj_q
runs entirely before attention (no filler tag).

The real limiter on this part is the HAM/thermal clock gate: sustained
high activity clamps PE to K=4/8 (1.2 GHz). v4 cuts scores PE cycles 2x,
batches the softmax-denominator reciprocals [4,512] (DVE's iterative
divide costs ~3.3us regardless of partition count), and moves the
normalization multiplies to the idle GPSIMD engine to cut total activity.

Softmax denominator comes from a ones-column appended to V (65th column of
each PV accumulator); normalization (reciprocal + DRAM-broadcast + mul)
runs off the critical path during the next pair's steps.
"""
import numpy as np
import ml_dtypes

import concourse.bass as bass
import concourse.bacc as bacc
import concourse.tile as tile
import concourse.mybir as mybir

N_CORES = 8
P = 128
B, S, D = 4, 2048, 1024
TOK = 1024  # my tokens
CD = D // P  # 8 chunks
NKC = S // P  # 16 key chunks
NHP = 8  # head pairs
F32 = mybir.dt.float32
BF16 = mybir.dt.bfloat16
EXP = mybir.ActivationFunctionType.Exp
PAIR_GROUPS = [[2 * i, 2 * i + 1] for i in range(4)]

# processing order of key chunks: V-arrival order (half0 both slots, then half1)
KC_ORDER = [0, 1, 2, 3, 8, 9, 10, 11, 4, 5, 6, 7, 12, 13, 14, 15]
L_PV = 1  # PV lag in pair-steps behind scores/exp

_CACHE = {}


def _n_excess_waits(nc):
    import json

    m = json.loads(nc.to_json_bytes())
    insts = [i for f in m["functions"] for b in f["blocks"] for i in b["instructions"]]
    return sum(
        1
        for i in insts
        if len((i.get("sync_info") or {}).get("on_wait", [])) >= 2
        and i.get("opcode") != "EventSemaphore"
    )


def _finish(nc):
    nc.compile()
    import bass_rust

    for _ in range(6):
        if _n_excess_waits(nc) == 0:
            break
        bass_rust.generate_event_semaphores(nc)
    assert _n_excess_waits(nc) == 0, "excess sync waits remain"
    nc.codegen_inst_isa_subclasses()
    return nc


def build_nc(scopes=False):
    nc = bacc.Bacc("TRN2", target_bir_lowering=False, debug=False, num_devices=N_CORES)

    xqT_d = nc.dram_tensor("xqT", [D, TOK], BF16, kind="ExternalInput").ap()
    xkT_d = nc.dram_tensor("xkT", [D, TOK], BF16, kind="ExternalInput").ap()
    xvT_d = nc.dram_tensor("xvT", [D, TOK], BF16, kind="ExternalInput").ap()
    # wk/wq chunk-major: [out-chunk i, partition p, (j, q)] with
    # wk4[i, p, j*128+q] = wk.T[j*128+p, i*128+q]
    wk4_d = nc.dram_tensor("wk4", [CD, P, D], BF16, kind="ExternalInput").ap()
    wq4_d = nc.dram_tensor("wq4", [CD, P, D], BF16, kind="ExternalInput").ap()
    wvT_d = nc.dram_tensor("wvT", [D, D], BF16, kind="ExternalInput").ap()
    woT_d = nc.dram_tensor("woT", [D, D], BF16, kind="ExternalInput").ap()
    out = nc.dram_tensor("out", [TOK, D], F32, kind="ExternalOutput").ap()

    # exchange buffers
    kag_i = nc.dram_tensor("kag_i", [D, TOK], BF16).ap()
    kag_oA = nc.dram_tensor("kag_oA", [2, D // 2, TOK], BF16).ap()  # d-chunks 0-3
    kag_oB = nc.dram_tensor("kag_oB", [2, D // 2, TOK], BF16).ap()  # d-chunks 4-7
    vag_i = nc.dram_tensor("vag_i", [TOK, D], BF16).ap()
    rec_d = nc.dram_tensor("rec_d", [32, 512], F32).ap()
    # V halves by my-token halves: vag_os[half][slot, tok512, d]
    vag_os = [
        nc.dram_tensor(f"vag_o{h}", [2, TOK // 2, D], BF16).ap() for h in range(2)
    ]

    from contextlib import ExitStack, nullcontext

    def scope(name):
        return nc.named_scope(name) if scopes else nullcontext()

    AG_KW = dict(
        kind="AllGather", op=mybir.AluOpType.bypass, replica_groups=PAIR_GROUPS
    )

    with tile.TileContext(nc) as tc:
        persist = ExitStack()
        qp = persist.enter_context(tc.tile_pool(name="qp", bufs=1))
        ltp = persist.enter_context(tc.tile_pool(name="ltp", bufs=1))
        kst = persist.enter_context(tc.tile_pool(name="kst", bufs=1))
        psp = persist.enter_context(tc.tile_pool(name="psp", bufs=1, space="PSUM"))
        stp = persist.enter_context(tc.tile_pool(name="stp", bufs=1))
        vp = persist.enter_context(tc.tile_pool(name="vp", bufs=1))
        pgp = persist.enter_context(tc.tile_pool(name="pgp", bufs=1))
        smp = persist.enter_context(tc.tile_pool(name="smp", bufs=1))
        arp = persist.enter_context(tc.tile_pool(name="arp", bufs=1))

        # close order is wvx -> wkx -> wqx, so create in reverse (pool stack is LIFO)
        wqx_stack = ExitStack()
        wqx = wqx_stack.enter_context(tc.tile_pool(name="wqx", bufs=1))
        wkx_stack = ExitStack()
        wkx = wkx_stack.enter_context(tc.tile_pool(name="wkx", bufs=1))
        wvx_stack = ExitStack()
        wvx = wvx_stack.enter_context(tc.tile_pool(name="wvx", bufs=1))

        with scope("warm"):
            wtile = smp.tile([128, 512], BF16, name="wtile")
            nc.vector.memset(wtile, 0.001)

        # pre-phase / wo psum tiles alternate between the two sg tags so that
        # chunk i+1's matmuls overlap chunk i's psum->sbuf copy.
        _ps_flip = [0]

        def ps_tile(name):
            _ps_flip[0] ^= 1
            tag = "sg01" if _ps_flip[0] else "sg23"
            return psp.tile([P, TOK], F32, name=name, tag=tag, bufs=1)

        # ---------------- loads (order matters: v, k, q) --------------------
        wk_c = {}  # streamed wk chunk tiles: i -> [128, 8, 128]
        wq_c = {}

        def load_wk_chunk(i):
            t = wkx.tile([P, CD, P], BF16, name=f"wkc_{i}", tag="wkc", bufs=3)
            nc.sync.dma_start(out=t, in_=wk4_d[i].rearrange("p (j q) -> p j q", q=P))
            wk_c[i] = t

        def load_wq_chunk(i):
            t = wqx.tile([P, CD, P], BF16, name=f"wqc_{i}", tag="wqc", bufs=3)
            nc.sync.dma_start(out=t, in_=wq4_d[i].rearrange("p (j q) -> p j q", q=P))
            wq_c[i] = t

        with scope("load"):
            xvT, wv_t, xkT, xqT = [], [], [], []
            for j in range(CD):
                t = wvx.tile([P, TOK], BF16, name=f"xvT_{j}")
                nc.sync.dma_start(out=t, in_=xvT_d[j * P : (j + 1) * P, :])
                xvT.append(t)
                w = wvx.tile([P, D], BF16, name=f"wv_{j}")
                nc.sync.dma_start(out=w, in_=wvT_d[j * P : (j + 1) * P, :])
                wv_t.append(w)
            for j in range(CD):
                t = wkx.tile([P, TOK], BF16, name=f"xkT_{j}")
                nc.sync.dma_start(out=t, in_=xkT_d[j * P : (j + 1) * P, :])
                xkT.append(t)
            load_wk_chunk(0)
            load_wk_chunk(1)
            for j in range(CD):
                t = wqx.tile([P, TOK], BF16, name=f"xqT_{j}")
                nc.sync.dma_start(out=t, in_=xqT_d[j * P : (j + 1) * P, :])
                xqT.append(t)
            load_wq_chunk(0)
            load_wq_chunk(1)

        def mm2(ps, lhsT, rhs, start, stop):
            # ISA caps matmul output at 512 fp32 elements (one PSUM bank):
            # emit two 512-wide matmuls covering a [*, 1024] psum tile
            for hh in range(2):
                nc.tensor.matmul(
                    ps[:, hh * 512 : (hh + 1) * 512],
                    lhsT,
                    rhs[:, hh * 512 : (hh + 1) * 512],
                    start=start,
                    stop=stop,
                )

        # ---------------- warmup spin: un-throttle PE HAM early -------------
        with scope("spin"):
            for k in range(24):
                ps = ps_tile("wm")
                nc.tensor.matmul(
                    ps[:, 0:512], wtile[:, 0:128], wtile, start=True, stop=True
                )

        # ---------------- phase 1a: proj_v t0-3 -----------------------------
        def proj_v_chunk(t_i, copy_eng):
            ps = ps_tile("ps_v")
            for j in range(CD):
                mm2(ps, xvT[j][:, t_i * P : (t_i + 1) * P], wv_t[j],
                    start=(j == 0), stop=(j == CD - 1))
            sb = stp.tile([P, D], BF16, name="sb_v", tag="st", bufs=3)
            copy_eng(sb, ps)
            nc.sync.dma_start(out=vag_i[t_i * P : (t_i + 1) * P, :], in_=sb)

        with scope("proj_v_a"):
            for t_i in range(4):
                proj_v_chunk(
                    t_i, nc.scalar.copy if t_i % 2 == 0 else nc.vector.tensor_copy
                )
        with scope("ag_v0"):
            nc.gpsimd.collective_compute(
                ins=[vag_i[0 : TOK // 2, :]], outs=[vag_os[0][:]], **AG_KW
            )
        with scope("proj_v_b"):
            for t_i in range(4, 8):
                proj_v_chunk(
                    t_i, nc.scalar.copy if t_i % 2 == 0 else nc.vector.tensor_copy
                )
        with scope("ag_v1"):
            nc.gpsimd.collective_compute(
                ins=[vag_i[TOK // 2 : TOK, :]], outs=[vag_os[1][:]], **AG_KW
            )
        wvx_stack.close()

        # ---------------- phase 1b: proj_k ----------------------------------
        def proj_k_chunk(i, copy_eng):
            if i + 2 < CD:
                load_wk_chunk(i + 2)
            ps = ps_tile("ps_k")
            wkc = wk_c.pop(i)
            for j in range(CD):
                mm2(ps, wkc[:, j, :], xkT[j], start=(j == 0), stop=(j == CD - 1))
            sb = stp.tile([P, TOK], BF16, name="sb_k", tag="st", bufs=3)
            copy_eng(sb, ps)
            nc.sync.dma_start(out=kag_i[i * P : (i + 1) * P, :], in_=sb)

        with scope("proj_k"):
            for i in range(CD):
                proj_k_chunk(i, nc.scalar.copy if i % 2 == 0 else nc.vector.tensor_copy)
                if i == 3:
                    with scope("ag_kA"):
                        nc.gpsimd.collective_compute(
                            ins=[kag_i[0 : D // 2, :]], outs=[kag_oA[:]], **AG_KW
                        )
        with scope("ag_kB"):
            nc.gpsimd.collective_compute(
                ins=[kag_i[D // 2 : D, :]], outs=[kag_oB[:]], **AG_KW
            )
        wkx_stack.close()

        # ---------------- K^T staging into SBUF -----------------------------
        kT_s = [None] * CD

        def kstage(j):
            t = kst.tile([P, S], BF16, name=f"kTs_{j}")
            kg = kag_oA if j < 4 else kag_oB
            jj = j % 4
            nc.sync.dma_start(out=t[:, 0:TOK], in_=kg[0, jj * P : (jj + 1) * P, :])
            nc.sync.dma_start(out=t[:, TOK:S], in_=kg[1, jj * P : (jj + 1) * P, :])
            kT_s[j] = t

        with scope("kstage"):
            for j in range(4):
                kstage(j)

        # ---------------- proj_q (all chunks, before attention) -------------
        qT_t = [None] * CD

        with scope("proj_q"):
            for i in range(CD):
                if i + 2 < CD:
                    load_wq_chunk(i + 2)
                psq = ps_tile("ps_q")
                wqc = wq_c.pop(i)
                for j in range(CD):
                    mm2(psq, wqc[:, j, :], xqT[j], start=(j == 0), stop=(j == CD - 1))
                qt = qp.tile([P, TOK], BF16, name=f"qT_{i}")
                (nc.scalar.copy if i % 2 == 0 else nc.vector.tensor_copy)(qt, psq)
                qT_t[i] = qt

        lts = [ltp.tile([P, TOK], BF16, name=f"lt_{i}") for i in range(CD)]

        # ---------------- attention (head pairs) ----------------------------
        steps = [(hp, pos) for hp in range(NHP) for pos in range(NKC)]
        n = len(steps)
        sgs, pgs, pvs, vts = {}, {}, {}, {}

        def load_head(h):
            v_t = vp.tile([P, NKC, 65], BF16, name="v_t", tag="vp", bufs=4)
            for half in range(2):
                for slot in range(2):
                    vsrc = vag_os[half][slot, :, 64 * h : 64 * h + 64]
                    base = 8 * half + 4 * slot
                    nc.sync.dma_start(
                        out=v_t[:, base : base + 4, 0:64],
                        in_=vsrc.rearrange("(kc p) d -> p kc d", p=P),
                    )
            nc.vector.memset(v_t[:, :, 64:65], 1.0)
            vts[h] = v_t
            # vts pos p: half=p//8, slot=(p%8)//4, i=p%4 -> kc = slot*8+half*4+i
            # which equals KC_ORDER[p].

        def emit_scores(s):
            hp, pos = steps[s]
            kc = KC_ORDER[pos]
            sg01 = psp.tile([P, TOK], F32, name="sg01", tag="sg01", bufs=1)
            sg23 = psp.tile([P, TOK], F32, name="sg23", tag="sg23", bufs=1)
            kA = kT_s[hp][0:64, kc * P : (kc + 1) * P]
            kB = kT_s[hp][64:128, kc * P : (kc + 1) * P]
            qA = qT_t[hp][0:64, :]
            qB = qT_t[hp][64:128, :]
            # A/B alternate row groups -> concurrent streaming on the PE array
            nc.tensor.matmul(sg01[:, 0:512], kA, qA[:, 0:512], start=True, stop=True)
            nc.tensor.matmul(sg01[:, 512:1024], kB, qB[:, 0:512], start=True, stop=True)
            nc.tensor.matmul(sg23[:, 0:512], kA, qA[:, 512:1024], start=True, stop=True)
            nc.tensor.matmul(sg23[:, 512:1024], kB, qB[:, 512:1024], start=True, stop=True)
            sgs[s] = (sg01, sg23)

        def emit_exp(s):
            sg01, sg23 = sgs.pop(s)
            pg01 = pgp.tile([P, TOK], BF16, name="pg01", tag="pg", bufs=2 * (L_PV + 2))
            nc.scalar.activation(pg01, sg01, EXP, scale=0.125)
            pg23 = pgp.tile([P, TOK], BF16, name="pg23", tag="pg", bufs=2 * (L_PV + 2))
            nc.scalar.activation(pg23, sg23, EXP, scale=0.125)
            pgs[s] = (pg01, pg23)

        def emit_pv(s):
            hp, pos = steps[s]
            hA, hB = 2 * hp, 2 * hp + 1
            pg01, pg23 = pgs.pop(s)
            if pos == 0:
                for h in (hA, hB):
                    for qb in range(2):
                        pvs[(h, qb)] = psp.tile(
                            [65, 512], F32, name=f"pv_{h % 2}_{qb}", tag="pv", bufs=4
                        )
            start, stop = (pos == 0), (pos == NKC - 1)
            nc.tensor.matmul(pvs[(hA, 0)], vts[hA][:, pos, :], pg01[:, 0:512],
                             start=start, stop=stop)
            nc.tensor.matmul(pvs[(hB, 0)], vts[hB][:, pos, :], pg01[:, 512:1024],
                             start=start, stop=stop)
            nc.tensor.matmul(pvs[(hA, 1)], vts[hA][:, pos, :], pg23[:, 0:512],
                             start=start, stop=stop)
            nc.tensor.matmul(pvs[(hB, 1)], vts[hB][:, pos, :], pg23[:, 512:1024],
                             start=start, stop=stop)
            if pos == NKC - 1:
                del vts[hA], vts[hB]
                finish_pair(hp)

        def finish_pair(hp):
            # units: (head, qb) -> rec_d row 4*hp + u
            with scope(f"norm_p{hp}"):
                ars = []
                dgt = smp.tile([4, 512], F32, name="dgt", tag="dgt", bufs=1)
                for u, (h, qb) in enumerate(
                    [(2 * hp, 0), (2 * hp + 1, 0), (2 * hp, 1), (2 * hp + 1, 1)]
                ):
                    pv = pvs.pop((h, qb))
                    ar = arp.tile([65, 512], F32, name="ar", tag="ar", bufs=4)
                    nc.vector.tensor_copy(ar, pv)  # releases the pv psum bank
                    nc.sync.dma_start(out=dgt[u : u + 1, :], in_=ar[64:65, :])
                    ars.append((h, qb, ar))
                rsq4 = smp.tile([4, 512], F32, name="rsq4", tag="rsq", bufs=1)
                nc.vector.reciprocal(rsq4, dgt)
                for u, (h, qb, ar) in enumerate(ars):
                    row = 4 * hp + u
                    # 1:1 write->read pairs: the dep tracker only syncs the
                    # first reader of a multi-row DRAM write across DMA queues
                    nc.sync.dma_start(
                        out=rec_d[row : row + 1, :], in_=rsq4[u : u + 1, :]
                    )
                    bc = smp.tile([64, 512], F32, name="bc", tag="bc", bufs=4)
                    nc.sync.dma_start(
                        out=bc,
                        in_=bass.AP(
                            tensor=rec_d.tensor,
                            offset=row * 512,
                            ap=[[0, 64], [1, 512]],
                        ),
                    )
                    rr = slice(64 * (h % 2), 64 * (h % 2) + 64)
                    nc.vector.tensor_mul(
                        lts[hp][rr, qb * 512 : (qb + 1) * 512], ar[0:64, :], bc
                    )

        wo_t = []
        with scope("attn"):
            load_head(0)
            load_head(1)
            emit_scores(0)
            for s in range(n):
                emit_exp(s)
                if s >= L_PV:
                    emit_pv(s - L_PV)
                if s + 1 < n:
                    hp1, pos1 = steps[s + 1]
                    if pos1 == 12 and hp1 + 1 < NHP:
                        load_head(2 * (hp1 + 1))
                        load_head(2 * (hp1 + 1) + 1)
                    emit_scores(s + 1)
                if s == 24:
                    for j in range(4, CD):
                        kstage(j)
                elif s == 40:
                    wqx_stack.close()
                    wop = persist.enter_context(tc.tile_pool(name="wop", bufs=1))
                    for j in range(CD):
                        wt3 = wop.tile([P, D], BF16, name=f"wo_{j}")
                        nc.sync.dma_start(out=wt3, in_=woT_d[j * P : (j + 1) * P, :])
                        wo_t.append(wt3)
            for s in range(n - L_PV, n):
                emit_pv(s)

        # ---------------- phase 3: output projection ------------------------
        with scope("wo"):
            for t_i in range(CD):
                ps3 = ps_tile("ps3")
                for sc in range(CD):
                    mm2(ps3, lts[sc][:, t_i * P : (t_i + 1) * P], wo_t[sc],
                        start=(sc == 0), stop=(sc == CD - 1))
                ob = stp.tile([P, D], F32, name="ob", tag="ob", bufs=2)
                nc.vector.tensor_copy(ob, ps3)
                nc.sync.dma_start(out=out[t_i * P : (t_i + 1) * P, :], in_=ob)

        persist.close()

    return _finish(nc)


def _get_nc(scopes=False):
    key = ("nc", scopes)
    if key not in _CACHE:
        _CACHE[key] = build_nc(scopes)
    return _CACHE[key]


def _chunk_major(wT):
    # wT: [D, D] = w.T ; return [CD, P, D] with out[i, p, j*128+q] = wT[j*128+p, i*128+q]
    return np.ascontiguousarray(
        wT.reshape(CD, P, CD, P).transpose(2, 1, 0, 3).reshape(CD, P, D)
    )


def make_in_maps(query, key, value, wq, wk, wv, wo):
    qf = np.asarray(query, np.float32).reshape(B * S, D)
    kf = np.asarray(key, np.float32).reshape(B * S, D)
    vf = np.asarray(value, np.float32).reshape(B * S, D)
    wk4_h = _chunk_major(np.asarray(wk).T.astype(np.float32)).astype(ml_dtypes.bfloat16)
    wq4_h = _chunk_major(np.asarray(wq).T.astype(np.float32)).astype(ml_dtypes.bfloat16)
    wvT_h = np.ascontiguousarray(np.asarray(wv).T).astype(ml_dtypes.bfloat16)
    woT_h = np.ascontiguousarray(np.asarray(wo).T).astype(ml_dtypes.bfloat16)
    in_maps = []
    for c in range(N_CORES):
        sl = slice(c * TOK, (c + 1) * TOK)
        in_maps.append(
            {
                "xqT": np.ascontiguousarray(qf[sl].T).astype(ml_dtypes.bfloat16),
                "xkT": np.ascontiguousarray(kf[sl].T).astype(ml_dtypes.bfloat16),
                "xvT": np.ascontiguousarray(vf[sl].T).astype(ml_dtypes.bfloat16),
                "wk4": wk4_h,
                "wq4": wq4_h,
                "wvT": wvT_h,
                "woT": woT_h,
            }
        )
    return in_maps


def assemble(results):
    blocks = [results[c]["out"] for c in range(N_CORES)]
    return np.concatenate(blocks, 0).reshape(B, S, D).astype(np.float32)


def kernel(query, key, value, mask, wq, wk, wv, wo):
    # mask is all-False in this problem: softmax without masking.
    nc = _get_nc()
    in_maps = make_in_maps(query, key, value, wq, wk, wv, wo)
    from concourse.bass_utils import run_bass_kernel_spmd

    res = run_bass_kernel_spmd(nc, in_maps, list(range(N_CORES)))
    return assemble(res.results)


# revision 18
# speedup vs baseline: 1.4375x; 1.0010x over previous
"""MHA on 8 NeuronCores, v4: head-pair row-tiled attention, ACT-bound pipeline.

Core c owns token block c = (batch c//2, seq half c%2), 1024 tokens.

Key idea vs v3: the scores matmul has contraction = head_dim = 64, so two
heads (even head at SBUF rows 0-63, odd head at rows 64-127 of kT_s/qT_t)
can stream CONCURRENTLY on the PE array via row tiling (tile_position is
auto-inferred from base partitions; different row_grps overlap on HW).
Attention therefore processes HEAD PAIRS: per pair-step (hp, kc):
  - 4 scores matmuls (A/B alternating row groups, 2 psum tiles
    sg01=[A qb0|B qb0], sg23=[A qb1|B qb1]) — A+B pairs stream together.
  - 2 exps [128,1024] on ACT (the pipeline bottleneck at full clock).
  - 4 PV matmuls (lagged L pair-steps) into 4 pv psum accumulators.
PSUM: sg01 (2 banks) + sg23 (2) + pv 4x[65,512] (4) = 8 banks, so proj_q
runs entirely before attention (no filler tag).

The real limiter on this part is the HAM/thermal clock gate: sustained
high activity clamps PE to K=4/8 (1.2 GHz). v4 cuts scores PE cycles 2x,
batches the softmax-denominator reciprocals [4,512] (DVE's iterative
divide costs ~3.3us regardless of partition count), and moves the
normalization multiplies to the idle GPSIMD engine to cut total activity.

Softmax denominator comes from a ones-column appended to V (65th column of
each PV accumulator); normalization (reciprocal + DRAM-broadcast + mul)
runs off the critical path during the next pair's steps.
"""
import numpy as np
import ml_dtypes

import concourse.bass as bass
import concourse.bacc as bacc
import concourse.tile as tile
import concourse.mybir as mybir

N_CORES = 8
P = 128
B, S, D = 4, 2048, 1024
TOK = 1024  # my tokens
CD = D // P  # 8 chunks
NKC = S // P  # 16 key chunks
NHP = 8  # head pairs
F32 = mybir.dt.float32
BF16 = mybir.dt.bfloat16
EXP = mybir.ActivationFunctionType.Exp
PAIR_GROUPS = [[2 * i, 2 * i + 1] for i in range(4)]

# processing order of key chunks: V-arrival order (half0 both slots, then half1)
KC_ORDER = [0, 1, 2, 3, 8, 9, 10, 11, 4, 5, 6, 7, 12, 13, 14, 15]
L_PV = 1  # PV lag in pair-steps behind scores/exp

_CACHE = {}


def _n_excess_waits(nc):
    import json

    m = json.loads(nc.to_json_bytes())
    insts = [i for f in m["functions"] for b in f["blocks"] for i in b["instructions"]]
    return sum(
        1
        for i in insts
        if len((i.get("sync_info") or {}).get("on_wait", [])) >= 2
        and i.get("opcode") != "EventSemaphore"
    )


def _finish(nc):
    nc.compile()
    import bass_rust

    for _ in range(6):
        if _n_excess_waits(nc) == 0:
            break
        bass_rust.generate_event_semaphores(nc)
    assert _n_excess_waits(nc) == 0, "excess sync waits remain"
    nc.codegen_inst_isa_subclasses()
    return nc


def build_nc(scopes=False):
    nc = bacc.Bacc("TRN2", target_bir_lowering=False, debug=False, num_devices=N_CORES)

    xqT_d = nc.dram_tensor("xqT", [D, TOK], BF16, kind="ExternalInput").ap()
    xkT_d = nc.dram_tensor("xkT", [D, TOK], BF16, kind="ExternalInput").ap()
    xvT_d = nc.dram_tensor("xvT", [D, TOK], BF16, kind="ExternalInput").ap()
    # wk/wq chunk-major: [out-chunk i, partition p, (j, q)] with
    # wk4[i, p, j*128+q] = wk.T[j*128+p, i*128+q]
    wk4_d = nc.dram_tensor("wk4", [CD, P, D], BF16, kind="ExternalInput").ap()
    wq4_d = nc.dram_tensor("wq4", [CD, P, D], BF16, kind="ExternalInput").ap()
    wvT_d = nc.dram_tensor("wvT", [D, D], BF16, kind="ExternalInput").ap()
    woT_d = nc.dram_tensor("woT", [D, D], BF16, kind="ExternalInput").ap()
    out = nc.dram_tensor("out", [TOK, D], F32, kind="ExternalOutput").ap()

    # exchange buffers
    kag_i = nc.dram_tensor("kag_i", [D, TOK], BF16).ap()
    kag_oA = nc.dram_tensor("kag_oA", [2, D // 2, TOK], BF16).ap()  # d-chunks 0-3
    kag_oB = nc.dram_tensor("kag_oB", [2, D // 2, TOK], BF16).ap()  # d-chunks 4-7
    vag_i = nc.dram_tensor("vag_i", [TOK, D], BF16).ap()
    rec_d = nc.dram_tensor("rec_d", [32, 512], F32).ap()
    # V halves by my-token halves: vag_os[half][slot, tok512, d]
    vag_os = [
        nc.dram_tensor(f"vag_o{h}", [2, TOK // 2, D], BF16).ap() for h in range(2)
    ]

    from contextlib import ExitStack, nullcontext

    def scope(name):
        return nc.named_scope(name) if scopes else nullcontext()

    AG_KW = dict(
        kind="AllGather", op=mybir.AluOpType.bypass, replica_groups=PAIR_GROUPS
    )

    with tile.TileContext(nc) as tc:
        persist = ExitStack()
        qp = persist.enter_context(tc.tile_pool(name="qp", bufs=1))
        ltp = persist.enter_context(tc.tile_pool(name="ltp", bufs=1))
        kst = persist.enter_context(tc.tile_pool(name="kst", bufs=1))
        psp = persist.enter_context(tc.tile_pool(name="psp", bufs=1, space="PSUM"))
        stp = persist.enter_context(tc.tile_pool(name="stp", bufs=1))
        vp = persist.enter_context(tc.tile_pool(name="vp", bufs=1))
        pgp = persist.enter_context(tc.tile_pool(name="pgp", bufs=1))
        smp = persist.enter_context(tc.tile_pool(name="smp", bufs=1))
        arp = persist.enter_context(tc.tile_pool(name="arp", bufs=1))

        # close order is wvx -> wkx -> wqx, so create in reverse (pool stack is LIFO)
        wqx_stack = ExitStack()
        wqx = wqx_stack.enter_context(tc.tile_pool(name="wqx", bufs=1))
        wkx_stack = ExitStack()
        wkx = wkx_stack.enter_context(tc.tile_pool(name="wkx", bufs=1))
        wvx_stack = ExitStack()
        wvx = wvx_stack.enter_context(tc.tile_pool(name="wvx", bufs=1))

        with scope("warm"):
            wtile = smp.tile([128, 512], BF16, name="wtile")
            nc.vector.memset(wtile, 0.001)

        # pre-phase / wo psum tiles alternate between the two sg tags so that
        # chunk i+1's matmuls overlap chunk i's psum->sbuf copy.
        _ps_flip = [0]

        def ps_tile(name):
            _ps_flip[0] ^= 1
            tag = "sg01" if _ps_flip[0] else "sg23"
            return psp.tile([P, TOK], F32, name=name, tag=tag, bufs=1)

        # ---------------- loads (order matters: v, k, q) --------------------
        wk_c = {}  # streamed wk chunk tiles: i -> [128, 8, 128]
        wq_c = {}

        def load_wk_chunk(i):
            t = wkx.tile([P, CD, P], BF16, name=f"wkc_{i}", tag="wkc", bufs=3)
            nc.sync.dma_start(out=t, in_=wk4_d[i].rearrange("p (j q) -> p j q", q=P))
            wk_c[i] = t

        def load_wq_chunk(i):
            t = wqx.tile([P, CD, P], BF16, name=f"wqc_{i}", tag="wqc", bufs=3)
            nc.sync.dma_start(out=t, in_=wq4_d[i].rearrange("p (j q) -> p j q", q=P))
            wq_c[i] = t

        with scope("load"):
            xvT, wv_t, xkT, xqT = [], [], [], []
            for j in range(CD):
                t = wvx.tile([P, TOK], BF16, name=f"xvT_{j}")
                nc.sync.dma_start(out=t, in_=xvT_d[j * P : (j + 1) * P, :])
                xvT.append(t)
                w = wvx.tile([P, D], BF16, name=f"wv_{j}")
                nc.sync.dma_start(out=w, in_=wvT_d[j * P : (j + 1) * P, :])
                wv_t.append(w)
            for j in range(CD):
                t = wkx.tile([P, TOK], BF16, name=f"xkT_{j}")
                nc.sync.dma_start(out=t, in_=xkT_d[j * P : (j + 1) * P, :])
                xkT.append(t)
            load_wk_chunk(0)
            load_wk_chunk(1)
            for j in range(CD):
                t = wqx.tile([P, TOK], BF16, name=f"xqT_{j}")
                nc.sync.dma_start(out=t, in_=xqT_d[j * P : (j + 1) * P, :])
                xqT.append(t)
            load_wq_chunk(0)
            load_wq_chunk(1)

        def mm2(ps, lhsT, rhs, start, stop):
            # ISA caps matmul output at 512 fp32 elements (one PSUM bank):
            # emit two 512-wide matmuls covering a [*, 1024] psum tile
            for hh in range(2):
                nc.tensor.matmul(
                    ps[:, hh * 512 : (hh + 1) * 512],
                    lhsT,
                    rhs[:, hh * 512 : (hh + 1) * 512],
                    start=start,
                    stop=stop,
                )

        # ---------------- warmup spin: un-throttle PE HAM early -------------
        with scope("spin"):
            for k in range(24):
                ps = ps_tile("wm")
                nc.tensor.matmul(
                    ps[:, 0:512], wtile[:, 0:128], wtile, start=True, stop=True
                )

        # ---------------- phase 1a: proj_v t0-3 -----------------------------
        def proj_v_chunk(t_i, copy_eng):
            ps = ps_tile("ps_v")
            for j in range(CD):
                mm2(ps, xvT[j][:, t_i * P : (t_i + 1) * P], wv_t[j],
                    start=(j == 0), stop=(j == CD - 1))
            sb = stp.tile([P, D], BF16, name="sb_v", tag="st", bufs=6)
            copy_eng(sb, ps)
            nc.sync.dma_start(out=vag_i[t_i * P : (t_i + 1) * P, :], in_=sb)

        with scope("proj_v_a"):
            for t_i in range(4):
                proj_v_chunk(
                    t_i, nc.scalar.copy if t_i % 2 == 0 else nc.vector.tensor_copy
                )
        with scope("ag_v0"):
            nc.gpsimd.collective_compute(
                ins=[vag_i[0 : TOK // 2, :]], outs=[vag_os[0][:]], **AG_KW
            )
        with scope("proj_v_b"):
            for t_i in range(4, 8):
                proj_v_chunk(
                    t_i, nc.scalar.copy if t_i % 2 == 0 else nc.vector.tensor_copy
                )
        with scope("ag_v1"):
            nc.gpsimd.collective_compute(
                ins=[vag_i[TOK // 2 : TOK, :]], outs=[vag_os[1][:]], **AG_KW
            )
        wvx_stack.close()

        # ---------------- phase 1b: proj_k ----------------------------------
        def proj_k_chunk(i, copy_eng):
            if i + 2 < CD:
                load_wk_chunk(i + 2)
            ps = ps_tile("ps_k")
            wkc = wk_c.pop(i)
            for j in range(CD):
                mm2(ps, wkc[:, j, :], xkT[j], start=(j == 0), stop=(j == CD - 1))
            sb = stp.tile([P, TOK], BF16, name="sb_k", tag="st", bufs=6)
            copy_eng(sb, ps)
            nc.sync.dma_start(out=kag_i[i * P : (i + 1) * P, :], in_=sb)

        with scope("proj_k"):
            for i in range(CD):
                proj_k_chunk(i, nc.scalar.copy if i % 2 == 0 else nc.vector.tensor_copy)
                if i == 3:
                    with scope("ag_kA"):
                        nc.gpsimd.collective_compute(
                            ins=[kag_i[0 : D // 2, :]], outs=[kag_oA[:]], **AG_KW
                        )
        with scope("ag_kB"):
            nc.gpsimd.collective_compute(
                ins=[kag_i[D // 2 : D, :]], outs=[kag_oB[:]], **AG_KW
            )
        wkx_stack.close()

        # ---------------- K^T staging into SBUF -----------------------------
        kT_s = [None] * CD

        def kstage(j):
            t = kst.tile([P, S], BF16, name=f"kTs_{j}")
            kg = kag_oA if j < 4 else kag_oB
            jj = j % 4
            nc.sync.dma_start(out=t[:, 0:TOK], in_=kg[0, jj * P : (jj + 1) * P, :])
            nc.sync.dma_start(out=t[:, TOK:S], in_=kg[1, jj * P : (jj + 1) * P, :])
            kT_s[j] = t

        with scope("kstage"):
            for j in range(4):
                kstage(j)

        # ---------------- attention defs + V/K staging (DMAs overlap proj_q) -
        # ---------------- attention (head pairs) ----------------------------
        steps = [(hp, pos) for hp in range(NHP) for pos in range(NKC)]
        n = len(steps)
        sgs, pgs, pvs, vts = {}, {}, {}, {}

        def load_head(h):
            v_t = vp.tile([P, NKC, 65], BF16, name="v_t", tag="vp", bufs=4)
            for half in range(2):
                for slot in range(2):
                    vsrc = vag_os[half][slot, :, 64 * h : 64 * h + 64]
                    base = 8 * half + 4 * slot
                    nc.sync.dma_start(
                        out=v_t[:, base : base + 4, 0:64],
                        in_=vsrc.rearrange("(kc p) d -> p kc d", p=P),
                    )
            nc.vector.memset(v_t[:, :, 64:65], 1.0)
            vts[h] = v_t
            # vts pos p: half=p//8, slot=(p%8)//4, i=p%4 -> kc = slot*8+half*4+i
            # which equals KC_ORDER[p].

        def emit_scores(s):
            hp, pos = steps[s]
            kc = KC_ORDER[pos]
            sg01 = psp.tile([P, TOK], F32, name="sg01", tag="sg01", bufs=1)
            sg23 = psp.tile([P, TOK], F32, name="sg23", tag="sg23", bufs=1)
            kA = kT_s[hp][0:64, kc * P : (kc + 1) * P]
            kB = kT_s[hp][64:128, kc * P : (kc + 1) * P]
            qA = qT_t[hp][0:64, :]
            qB = qT_t[hp][64:128, :]
            # A/B alternate row groups -> concurrent streaming on the PE array
            nc.tensor.matmul(sg01[:, 0:512], kA, qA[:, 0:512], start=True, stop=True)
            nc.tensor.matmul(sg01[:, 512:1024], kB, qB[:, 0:512], start=True, stop=True)
            nc.tensor.matmul(sg23[:, 0:512], kA, qA[:, 512:1024], start=True, stop=True)
            nc.tensor.matmul(sg23[:, 512:1024], kB, qB[:, 512:1024], start=True, stop=True)
            sgs[s] = (sg01, sg23)

        def emit_exp(s):
            sg01, sg23 = sgs.pop(s)
            pg01 = pgp.tile([P, TOK], BF16, name="pg01", tag="pg", bufs=2 * (L_PV + 2))
            nc.scalar.activation(pg01, sg01, EXP, scale=0.125)
            pg23 = pgp.tile([P, TOK], BF16, name="pg23", tag="pg", bufs=2 * (L_PV + 2))
            nc.scalar.activation(pg23, sg23, EXP, scale=0.125)
            pgs[s] = (pg01, pg23)

        def emit_pv(s):
            hp, pos = steps[s]
            hA, hB = 2 * hp, 2 * hp + 1
            pg01, pg23 = pgs.pop(s)
            if pos == 0:
                for h in (hA, hB):
                    for qb in range(2):
                        pvs[(h, qb)] = psp.tile(
                            [65, 512], F32, name=f"pv_{h % 2}_{qb}", tag="pv", bufs=4
                        )
            start, stop = (pos == 0), (pos == NKC - 1)
            nc.tensor.matmul(pvs[(hA, 0)], vts[hA][:, pos, :], pg01[:, 0:512],
                             start=start, stop=stop)
            nc.tensor.matmul(pvs[(hB, 0)], vts[hB][:, pos, :], pg01[:, 512:1024],
                             start=start, stop=stop)
            nc.tensor.matmul(pvs[(hA, 1)], vts[hA][:, pos, :], pg23[:, 0:512],
                             start=start, stop=stop)
            nc.tensor.matmul(pvs[(hB, 1)], vts[hB][:, pos, :], pg23[:, 512:1024],
                             start=start, stop=stop)
            if pos == NKC - 1:
                del vts[hA], vts[hB]
                finish_pair(hp)

        def finish_pair(hp):
            # units: (head, qb) -> rec_d row 4*hp + u
            with scope(f"norm_p{hp}"):
                ars = []
                dgt = smp.tile([4, 512], F32, name="dgt", tag="dgt", bufs=1)
                units = [(2 * hp, 0), (2 * hp + 1, 0), (2 * hp, 1), (2 * hp + 1, 1)]
                for u, (h, qb) in enumerate(units):
                    pv = pvs.pop((h, qb))
                    ar = arp.tile([65, 512], F32, name="ar", tag="ar", bufs=4)
                    nc.vector.tensor_copy(ar, pv)  # releases the pv psum bank
                    nc.sync.dma_start(out=dgt[u : u + 1, :], in_=ar[64:65, :])
                    ars.append((h, qb, ar))
                rsq4 = smp.tile([4, 512], F32, name="rsq4", tag="rsq", bufs=1)
                nc.vector.reciprocal(rsq4, dgt)
                # single 1:1 write->read DRAM roundtrip for the broadcast
                nc.sync.dma_start(out=rec_d[4 * hp : 4 * hp + 4, :], in_=rsq4)
                bc4 = smp.tile([64, 4, 512], F32, name="bc4", tag="bc", bufs=1)
                nc.sync.dma_start(
                    out=bc4,
                    in_=bass.AP(
                        tensor=rec_d.tensor,
                        offset=4 * hp * 512,
                        ap=[[0, 64], [512, 4], [1, 512]],
                    ),
                )
                for u, (h, qb, ar) in enumerate(ars):
                    rr = slice(64 * (h % 2), 64 * (h % 2) + 64)
                    nc.vector.tensor_mul(
                        lts[hp][rr, qb * 512 : (qb + 1) * 512],
                        ar[0:64, :],
                        bc4[:, u, :],
                    )
        with scope("vstage"):
            for j in range(4, CD):
                kstage(j)
            for h in range(4):
                load_head(h)

        # ---------------- proj_q (all chunks, before attention) -------------
        qT_t = [None] * CD

        with scope("proj_q"):
            for i in range(CD):
                if i + 2 < CD:
                    load_wq_chunk(i + 2)
                psq = ps_tile("ps_q")
                wqc = wq_c.pop(i)
                for j in range(CD):
                    mm2(psq, wqc[:, j, :], xqT[j], start=(j == 0), stop=(j == CD - 1))
                qt = qp.tile([P, TOK], BF16, name=f"qT_{i}")
                (nc.scalar.copy if i % 2 == 0 else nc.vector.tensor_copy)(qt, psq)
                qT_t[i] = qt

        lts = [ltp.tile([P, TOK], BF16, name=f"lt_{i}") for i in range(CD)]

        wo_t = []
        with scope("attn"):
            emit_scores(0)
            for s in range(n):
                emit_exp(s)
                if s >= L_PV:
                    emit_pv(s - L_PV)
                if s + 1 < n:
                    hp1, pos1 = steps[s + 1]
                    if pos1 == 12 and hp1 + 1 < NHP and 2 * (hp1 + 1) not in vts:
                        load_head(2 * (hp1 + 1))
                        load_head(2 * (hp1 + 1) + 1)
                    emit_scores(s + 1)
                if s == 40:
                    wqx_stack.close()
                    wop = persist.enter_context(tc.tile_pool(name="wop", bufs=1))
                    for j in range(CD):
                        wt3 = wop.tile([P, D], BF16, name=f"wo_{j}")
                        nc.sync.dma_start(out=wt3, in_=woT_d[j * P : (j + 1) * P, :])
                        wo_t.append(wt3)
            for s in range(n - L_PV, n):
                emit_pv(s)

        # ---------------- phase 3: output projection ------------------------
        # t0/t1 accumulate sc=0..6 immediately (no deps) to cover the last
        # finish_pair chain; tailwarm spins on the freed pv banks keep the
        # PE's HAM activity window busy so wo runs at full clock.
        with scope("wo"):
            ps3s = {}
            for t_i in (0, 1):
                ps3 = ps_tile("ps3")
                for sc in range(CD - 1):
                    mm2(ps3, lts[sc][:, t_i * P : (t_i + 1) * P], wo_t[sc],
                        start=(sc == 0), stop=False)
                ps3s[t_i] = ps3
            with scope("tailwarm"):
                for k in range(10):
                    pw = psp.tile([65, 512], F32, name="pw", tag="pv", bufs=4)
                    nc.tensor.matmul(pw, wtile[:, 0:65], wtile, start=True, stop=True)
            for t_i in range(CD):
                if t_i < 2:
                    ps3 = ps3s[t_i]
                    mm2(ps3, lts[CD - 1][:, t_i * P : (t_i + 1) * P], wo_t[CD - 1],
                        start=False, stop=True)
                else:
                    ps3 = ps_tile("ps3")
                    for sc in range(CD):
                        mm2(ps3, lts[sc][:, t_i * P : (t_i + 1) * P], wo_t[sc],
                            start=(sc == 0), stop=(sc == CD - 1))
                ob = stp.tile([P, D], F32, name="ob", tag="ob", bufs=2)
                nc.vector.tensor_copy(ob, ps3)
                nc.sync.dma_start(out=out[t_i * P : (t_i + 1) * P, :], in_=ob)

        persist.close()

    return _finish(nc)


def _get_nc(scopes=False):
    key = ("nc", scopes)
    if key not in _CACHE:
        _CACHE[key] = build_nc(scopes)
    return _CACHE[key]


def _chunk_major(wT):
    # wT: [D, D] = w.T ; return [CD, P, D] with out[i, p, j*128+q] = wT[j*128+p, i*128+q]
    return np.ascontiguousarray(
        wT.reshape(CD, P, CD, P).transpose(2, 1, 0, 3).reshape(CD, P, D)
    )


def make_in_maps(query, key, value, wq, wk, wv, wo):
    qf = np.asarray(query, np.float32).reshape(B * S, D)
    kf = np.asarray(key, np.float32).reshape(B * S, D)
    vf = np.asarray(value, np.float32).reshape(B * S, D)
    wk4_h = _chunk_major(np.asarray(wk).T.astype(np.float32)).astype(ml_dtypes.bfloat16)
    wq4_h = _chunk_major(np.asarray(wq).T.astype(np.float32)).astype(ml_dtypes.bfloat16)
    wvT_h = np.ascontiguousarray(np.asarray(wv).T).astype(ml_dtypes.bfloat16)
    woT_h = np.ascontiguousarray(np.asarray(wo).T).astype(ml_dtypes.bfloat16)
    in_maps = []
    for c in range(N_CORES):
        sl = slice(c * TOK, (c + 1) * TOK)
        in_maps.append(
            {
                "xqT": np.ascontiguousarray(qf[sl].T).astype(ml_dtypes.bfloat16),
                "xkT": np.ascontiguousarray(kf[sl].T).astype(ml_dtypes.bfloat16),
                "xvT": np.ascontiguousarray(vf[sl].T).astype(ml_dtypes.bfloat16),
                "wk4": wk4_h,
                "wq4": wq4_h,
                "wvT": wvT_h,
                "woT": woT_h,
            }
        )
    return in_maps


def assemble(results):
    blocks = [results[c]["out"] for c in range(N_CORES)]
    return np.concatenate(blocks, 0).reshape(B, S, D).astype(np.float32)


def kernel(query, key, value, mask, wq, wk, wv, wo):
    # mask is all-False in this problem: softmax without masking.
    nc = _get_nc()
    in_maps = make_in_maps(query, key, value, wq, wk, wv, wo)
    from concourse.bass_utils import run_bass_kernel_spmd

    res = run_bass_kernel_spmd(nc, in_maps, list(range(N_CORES)))
    return assemble(res.results)
